# revision 56
# baseline (speedup 1.0000x reference)
"""Bass/Trainium2 kernel for causal attention with memory prefix (SPMD on 8 NeuronCores).

Problem shapes (hardcoded): b=2, n=2048, mem=512, dim=1024, heads=16, d=64.
Sharding: core c handles batch c//4 and head-group c%4 (4 heads = 256 dims).
Each core computes q/k/v projections for its head group (bf16 inputs), causal
attention in transposed (scoresT = [j, i]) layout with fp32r matmuls, and a
partial output projection. The output is reduced with 4 chunked
ReduceScatter(add) collectives over each 4-core group (overlapped with
compute); each core ends with 4 distinct 128-row chunks of its batch's output,
which the host reassembles.
"""

import numpy as np
import ml_dtypes

import concourse.bacc as bacc
import concourse.mybir as mybir
import concourse.tile as tile
from concourse.bass_utils import run_bass_kernel_spmd

N_CORES = 8
B, N, M, DIM = 2, 2048, 512, 1024
H, D = 16, 64
J = M + N  # 2560
HG = 4  # heads per core
HD = HG * D  # 256 head-group width
NSEG = J // 512  # 5 segments of the j axis
NJT = J // 128  # 20 j-tiles
NIB = N // 512  # 4 i-blocks

F32 = mybir.dt.float32
F32R = mybir.dt.float32r
BF16 = mybir.dt.bfloat16


def build():
    nc = bacc.Bacc("TRN2", target_bir_lowering=False, debug=False, num_devices=N_CORES)

    # host-packed layouts: partition dim first everywhere
    XCT = nc.dram_tensor("xct", [NSEG, 128, 8 * 512], BF16, kind="ExternalInput")
    WQ = nc.dram_tensor("wq", [128, 8 * HD], BF16, kind="ExternalInput")
    WK = nc.dram_tensor("wk", [128, 8 * HD], BF16, kind="ExternalInput")
    WV = nc.dram_tensor("wv", [128, 8 * HD], BF16, kind="ExternalInput")
    WOUT = nc.dram_tensor("wout", [128, 2 * DIM], F32R, kind="ExternalInput")
    BQ = nc.dram_tensor("bq", [1, DIM], F32R, kind="ExternalInput")
    MASKS = nc.dram_tensor("masks", [128, 128], BF16, kind="ExternalInput")
    ONESC = nc.dram_tensor("onesc", [128, NJT * HG], F32R, kind="ExternalInput")
    ONESR = nc.dram_tensor("onesr", [1, 128], F32R, kind="ExternalInput")
    OUT = nc.dram_tensor("out", [N // 4, DIM], F32, kind="ExternalOutput")

    with tile.TileContext(nc) as tc:
        with (
            tc.tile_pool(name="persist", bufs=1) as pp,
            tc.tile_pool(name="xs_pool", bufs=2) as xsp,
            tc.tile_pool(name="exp_pool", bufs=12) as ep,
            tc.tile_pool(name="norm_pool", bufs=3) as np_,
            tc.tile_pool(name="y_pool", bufs=4) as yp,
                        tc.tile_pool(name="ps_acc", bufs=4, space="PSUM") as psa,
            tc.tile_pool(name="ps_sc", bufs=2, space="PSUM") as pss,
            tc.tile_pool(name="dram", bufs=1, space="DRAM") as dp,
        ):
            # ---- persistent SBUF tensors ----
            wq_sb = pp.tile([128, 8 * HD], BF16, tag="wq")
            wk_sb = pp.tile([128, 8 * HD], BF16, tag="wk")
            wv_sb = pp.tile([128, 8 * HD], BF16, tag="wv")
            wout_sb = pp.tile([128, 2 * DIM], F32R, tag="wout")
            bq_sb = pp.tile([1, DIM], F32R, tag="bq")
            mask_sb = pp.tile([128, 128], BF16, tag="mask")
            onesr_sb = pp.tile([1, 128], F32R, tag="onesr")
            kt_sb = pp.tile([128, 2 * J], F32R, tag="kt")  # [d-pair(128), pair*2560]
            qt_sb = pp.tile([128, 2 * N], F32R, tag="qt")  # [d-pair(128), pair*2048]
            v_sb = pp.tile([128, NJT * (HG * 65)], F32R, tag="v")  # per jt: 4 heads x 65
            oh_sb = [pp.tile([128, N], F32R, tag=f"oh{i}", name=f"oh{i}") for i in range(2)]

            nc.sync.dma_start(out=wk_sb[:], in_=WK[:])

            # ---- Phase A: projections as a job queue, interleaved into the
            # attention stream below (one job ~= one PSUM accumulation group).
            xs_tiles = {}

            def load_seg(s):
                xs = xsp.tile([128, 8 * 512], BF16, tag="xs", name="xs")
                nc.sync.dma_start(out=xs[:], in_=XCT[s])
                xs_tiles[s] = xs
                if s == 0:
                    nc.sync.dma_start(out=wq_sb[:], in_=WQ[:])
                    nc.sync.dma_start(out=wv_sb[:], in_=WV[:])
                    nc.sync.dma_start(out=wout_sb[:], in_=WOUT[:])
                    nc.sync.dma_start(out=bq_sb[:], in_=BQ[:])
                    nc.sync.dma_start(out=mask_sb[:], in_=MASKS[:])
                    nc.sync.dma_start(out=onesr_sb[:], in_=ONESR[:])
                    # ones columns of v_ext: one strided DMA
                    nc.sync.dma_start(
                        out=v_sb[:].rearrange(
                            "p (t h e) -> p t h e", t=NJT, h=HG
                        )[:, :, :, 64],
                        in_=ONESC[:].rearrange("p (t h) -> p t h", t=NJT),
                    )

            def kq_chunk(s, p, w_sb, dst, dst_off):
                xs = xs_tiles[s]
                acc = psa.tile([128, 512], F32, tag="acc", name="acckq")
                for cc in range(8):
                    nc.tensor.matmul(
                        acc[:],
                        w_sb[:, HD * cc + 128 * p : HD * cc + 128 * p + 128],
                        xs[:, 512 * cc : 512 * cc + 512],
                        start=(cc == 0),
                        stop=(cc == 7),
                    )
                nc.vector.tensor_copy(dst[:, dst_off : dst_off + 512], acc[:])

            def v_chunk(s, jc):
                xs = xs_tiles[s]
                jt = 4 * s + jc
                acc = psa.tile([128, 512], F32, tag="acc", name="accv")
                for cc in range(8):
                    nc.tensor.matmul(
                        acc[:, 0:HD],
                        xs[:, 512 * cc + 128 * jc : 512 * cc + 128 * jc + 128],
                        wv_sb[:, HD * cc : HD * cc + HD],
                        start=(cc == 0),
                        stop=(cc == 7),
                    )
                nc.vector.tensor_copy(
                    v_sb[:, 65 * HG * jt : 65 * HG * (jt + 1)].rearrange(
                        "p (h e) -> p h e", h=HG
                    )[:, :, 0:64],
                    acc[:, 0:HD].rearrange("p (h e) -> p h e", h=HG),
                )

            proj_jobs = []  # (segment, fn, args)
            for s in range(NSEG):
                proj_jobs.append((s, load_seg, (s,)))
                for p in range(2):
                    proj_jobs.append((s, kq_chunk, (s, p, wk_sb, kt_sb, J * p + 512 * s)))
                if s >= 1:
                    for p in range(2):
                        proj_jobs.append(
                            (s, kq_chunk, (s, p, wq_sb, qt_sb, N * p + 512 * (s - 1)))
                        )
                for jc in range(4):
                    proj_jobs.append((s, v_chunk, (s, jc)))

            def drain_proj(upto_seg):
                while proj_jobs and proj_jobs[0][0] <= upto_seg:
                    _, fn, args = proj_jobs.pop(0)
                    fn(*args)

            def pop_proj_one():
                if proj_jobs:
                    _, fn, args = proj_jobs.pop(0)
                    fn(*args)
                    return True
                return False

            # ---- Phases B+C+D: one global software pipeline over (ib, h) ----
            yb = [dp.tile([512, DIM], BF16, name=f"ybounce{ib}") for ib in range(NIB)]
            yrs = [dp.tile([128, DIM], BF16, name=f"yrs{ib}") for ib in range(NIB)]
            DEPTH = 8
            oht_tiles = {}
            pend = []  # (ib, h, jtp, ex)
            actions = []  # deferred emissions (one per main-loop step)

            def emit_av(ib, h, jtp, ex):
                jt_hi = 4 * ib + 7
                oht = oht_tiles[(ib, h)]
                for d in (0, 1):
                    jt = jtp + d
                    o = max(0, 128 * (jt - (4 * ib + 4)))
                    nc.tensor.matmul(
                        oht[0:65, o:512],
                        v_sb[:, 65 * HG * jt + 65 * h : 65 * HG * jt + 65 * h + 65],
                        ex[:, 512 * d + o : 512 * d + 512],
                        start=(jt == 0),
                        stop=(jt == jt_hi),
                    )
                if jtp + 1 == jt_hi:
                    emit_norm(ib, h)
                    if h == HG - 1:
                        for itl in range(4):
                            actions.append((emit_outproj, (ib, itl)))
                        actions.append((emit_rs, (ib,)))

            def emit_norm(ib, h):
                p, l = h // 2, h % 2
                oht = oht_tiles.pop((ib, h))
                s_sb = np_.tile([1, 512], F32, tag="s", name="s_sb")
                nc.vector.tensor_copy(s_sb[:], oht[64:65, :])
                bc = np_.tile([64, 512], F32, tag="bc", name="bc")
                nc.gpsimd.partition_broadcast(bc[:], s_sb[:])
                rec = np_.tile([64, 512], F32, tag="rec", name="rec")
                nc.vector.reciprocal(rec[:], bc[:])
                if l == 0:
                    nc.vector.tensor_tensor(
                        oh_sb[p][0:64, 512 * ib : 512 * ib + 512],
                        oht[0:64, :],
                        rec[:],
                        mybir.AluOpType.mult,
                    )
                else:
                    tmp = np_.tile([64, 512], F32R, tag="tmp", name="tmp")
                    nc.vector.tensor_tensor(
                        tmp[:], oht[0:64, :], rec[:], mybir.AluOpType.mult
                    )
                    nc.sync.dma_start(
                        out=oh_sb[p][64:128, 512 * ib : 512 * ib + 512],
                        in_=tmp[:],
                    )

            def emit_outproj(ib, itl):
                it = 4 * ib + itl
                ysb = yp.tile([128, DIM], BF16, tag="y", name="ysb")
                for eb in range(2):
                    yps = psa.tile([128, 512], F32, tag="acc", name="yps")
                    nc.tensor.matmul(
                        yps[:],
                        oh_sb[0][:, 128 * it : 128 * it + 128],
                        wout_sb[:, 512 * eb : 512 * eb + 512],
                        start=True,
                        stop=False,
                    )
                    nc.tensor.matmul(
                        yps[:],
                        oh_sb[1][:, 128 * it : 128 * it + 128],
                        wout_sb[:, DIM + 512 * eb : DIM + 512 * eb + 512],
                        start=False,
                        stop=False,
                    )
                    nc.tensor.matmul(
                        yps[:],
                        onesr_sb[:],
                        bq_sb[:, 512 * eb : 512 * eb + 512],
                        start=False,
                        stop=True,
                    )
                    nc.vector.tensor_copy(ysb[:, 512 * eb : 512 * eb + 512], yps[:])
                nc.sync.dma_start(
                    out=yb[ib][128 * itl : 128 * itl + 128, :], in_=ysb[:]
                )

            def emit_rs(ib):
                nc.gpsimd.collective_compute(
                    "ReduceScatter",
                    mybir.AluOpType.add,
                    replica_groups=[[0, 1, 2, 3], [4, 5, 6, 7]],
                    ins=[yb[ib].opt()],
                    outs=[yrs[ib].opt()],
                )
                actions.append(
                    (
                        lambda ib=ib: nc.gpsimd.dma_start(
                            out=OUT[128 * ib : 128 * ib + 128, :], in_=yrs[ib][:]
                        ),
                        (),
                    )
                )

            for ib in range(NIB):
                jt_hi = 4 * ib + 7  # inclusive; range length is even
                drain_proj(ib + 1)
                for h in range(HG):
                    p, l = h // 2, h % 2
                    po = 64 * l
                    kt_h = kt_sb[po : po + 64, J * p : J * p + J]
                    qt_h = qt_sb[po : po + 64, N * p : N * p + N]
                    oht_tiles[(ib, h)] = psa.tile(
                        [128, 512], F32, tag="acc", name=f"oht{ib}_{h}"
                    )
                    for jtp in range(0, jt_hi + 1, 2):
                        # per-half column offset: diagonal tiles only need
                        # i >= o (o = 128 * (jt - 4*ib - 4)); left of that is
                        # fully masked and skipped outright.
                        offs = [
                            max(0, 128 * ((jtp + d) - (4 * ib + 4)))
                            for d in (0, 1)
                        ]
                        sc = pss.tile([128, 1024], F32, tag="sc", name="sc")
                        for d in (0, 1):
                            jt = jtp + d
                            o = offs[d]
                            nc.tensor.matmul(
                                sc[:, 512 * d + o : 512 * d + 512],
                                kt_h[:, 128 * jt : 128 * jt + 128],
                                qt_h[:, 512 * ib + o : 512 * ib + 512],
                                start=True,
                                stop=True,
                            )
                        ex = ep.tile([128, 1024], F32R, tag="ex", name="ex")
                        if offs[1] == 0:
                            nc.scalar.activation(
                                ex[:], sc[:], mybir.ActivationFunctionType.Exp
                            )
                        else:
                            for d in (0, 1):
                                o = offs[d]
                                nc.scalar.activation(
                                    ex[:, 512 * d + o : 512 * d + 512],
                                    sc[:, 512 * d + o : 512 * d + 512],
                                    mybir.ActivationFunctionType.Exp,
                                )
                        if jtp >= 4 * ib + 3:
                            # triangular strip mask on the first 128 valid
                            # columns of each diagonal half
                            for d in (0, 1):
                                if (jtp + d) - (4 * ib + 4) >= 0:
                                    o = offs[d]
                                    nc.vector.tensor_tensor(
                                        ex[:, 512 * d + o : 512 * d + o + 128],
                                        ex[:, 512 * d + o : 512 * d + o + 128],
                                        mask_sb[:],
                                        mybir.AluOpType.mult,
                                    )
                        pend.append((ib, h, jtp, ex))
                        if len(pend) > DEPTH:
                            emit_av(*pend.pop(0))
                        # filler work: projection chunks, then deferred actions
                        if len(oht_tiles) <= 2:
                            if not pop_proj_one() and actions:
                                fn, args = actions.pop(0)
                                fn(*args)
            while pend:
                emit_av(*pend.pop(0))
                if actions:
                    fn, args = actions.pop(0)
                    fn(*args)
            while actions:
                fn, args = actions.pop(0)
                fn(*args)

    nc.compile()
    return nc


def shard_inputs(x, mem, Wq, Wkv, Wout, bout):
    x = np.asarray(x, dtype=np.float32)
    mem = np.asarray(mem, dtype=np.float32)
    Wq = np.asarray(Wq, dtype=np.float32)
    Wkv = np.asarray(Wkv, dtype=np.float32)
    Wout = np.asarray(Wout, dtype=np.float32)
    bout = np.asarray(bout, dtype=np.float32)

    xc = np.concatenate([mem, x], axis=1)  # [B, J, DIM]

    # additive causal masks for the 4 diagonal offsets, packed [128, 4*512]
    jl = np.arange(128)[:, None]
    il = np.arange(512)[None, :]
    cl = np.arange(128)[None, :]
    masks = np.where(cl >= jl, 1.0, 0.0).astype(ml_dtypes.bfloat16)  # [128, 128]

    onesc = np.ones((128, NJT * HG), np.float32)
    onesr = np.ones((1, 128), np.float32)
    bq = (bout / 4.0).reshape(1, DIM)

    def pack_w(w):  # [1024, m] -> [128, 8*m]
        m = w.shape[1]
        return np.ascontiguousarray(
            w.reshape(8, 128, m).transpose(1, 0, 2).reshape(128, 8 * m)
        )

    in_maps = []
    for c in range(N_CORES):
        g, r = c // 4, c % 4
        xcT = xc[g].T  # [1024, 2560]
        # [s, p, 512k + j] = xcT[128k + p, 512s + j]
        x2 = (
            np.ascontiguousarray(
                xcT.reshape(8, 128, NSEG, 512)
                .transpose(2, 1, 0, 3)
                .reshape(NSEG, 128, 8 * 512)
            )
        ).astype(ml_dtypes.bfloat16)
        wq = pack_w(Wq[:, HD * r : HD * (r + 1)] * (D**-0.5)).astype(
            ml_dtypes.bfloat16
        )
        wk = pack_w(Wkv[:, HD * r : HD * (r + 1)]).astype(ml_dtypes.bfloat16)
        wv = pack_w(Wkv[:, DIM + HD * r : DIM + HD * (r + 1)]).astype(
            ml_dtypes.bfloat16
        )
        wout = np.ascontiguousarray(
            Wout[HD * r : HD * (r + 1), :]
            .reshape(2, 128, DIM)
            .transpose(1, 0, 2)
            .reshape(128, 2 * DIM)
        )
        in_maps.append(
            {
                "xct": x2,
                "wq": wq,
                "wk": wk,
                "wv": wv,
                "wout": wout,
                "bq": bq,
                "masks": masks,
                "onesc": onesc,
                "onesr": onesr,
            }
        )
    return in_maps


def assemble_output(results):
    out = np.empty((B, N, DIM), np.float32)
    for g in range(B):
        for r in range(4):
            chunk = results[4 * g + r]["out"]  # [512, 1024]: 4 x 128-row pieces
            for ib in range(NIB):
                out[g, 512 * ib + 128 * r : 512 * ib + 128 * (r + 1)] = chunk[
                    128 * ib : 128 * (ib + 1)
                ]
    return out


_NC_CACHE = None


def _get_nc():
    global _NC_CACHE
    if _NC_CACHE is None:
        _NC_CACHE = build()
    return _NC_CACHE


def kernel(x, mem, Wq, Wkv, Wout, bout):
    nc = _get_nc()
    in_maps = shard_inputs(x, mem, Wq, Wkv, Wout, bout)
    res = run_bass_kernel_spmd(nc, in_maps, core_ids=list(range(N_CORES)))
    return assemble_output(res.results)
